# revision 1
# baseline (speedup 1.0000x reference)
"""Trainium2 Bass kernel for nn_BiasedMultiHeadAttention (B=4, H=16, L=1024, E=1024).

Sharding: 64 (batch, head) pairs over 8 cores -> core c handles batch b=c//2,
heads h0=(c%2)*8 .. h0+8. Each core runs LayerNorm + its Q/K/V projection
slices + biased masked attention for its 8 heads + its slice of the output
projection (row-parallel). The two cores sharing a batch each return a partial
[L, E] out-projection; the host sums the pair and adds residual + bo.

Host-side folding (exact algebra, done in fp32):
  - gamma/beta folded into the projection weights/biases
  - 1/sqrt(D) folded into Wq/bq
  - gate*bias pre-exponentiated: device computes exp(Q K^T) * egb where
    egb = exp(gate*bias) * keymask * querymask (softmax shift/scale cancels in
    the normalization, and masking becomes an exact multiply-by-zero)
  - an epsilon row seeds the softmax denominator so fully-masked query columns
    normalize to exactly 0 instead of NaN.

Device layouts (per core): attention runs transposed, logitsT[k, q], so the
softmax denominator falls out of the attention*V matmul via an appended
ones-column on V, and the key mask rides for free inside egb.
"""
import numpy as np
import ml_dtypes
from contextlib import ExitStack

import concourse.bass as bass
import concourse.bacc as bacc
import concourse.tile as tile
from concourse import mybir
from concourse.bass_utils import run_bass_kernel_spmd

BF16 = mybir.dt.bfloat16
F32 = mybir.dt.float32
NBF16 = ml_dtypes.bfloat16
AF = mybir.ActivationFunctionType
ALU = mybir.AluOpType

P = 128
B, L, E, D, H = 4, 1024, 1024, 64, 16
HPC = 8            # heads per core
FL = HPC * D       # local feature width = 512
FC = FL // P       # 4 feature chunks
EC = E // P        # 8 embed chunks
LC = L // P        # 8 sequence chunks
NCORES = 8
LN_EPS = 1e-5

_NC = None


def _emit(nc, tc, ctx, xd, wq_d, wk_d, wv_d, wo_d, bq_d, bk_d, bv_d, eg_d, out_d):
    sync = nc.sync
    x_t = xd.ap().rearrange("(t p) e -> t p e", p=P)
    out_t = out_d.ap().rearrange("(t p) e -> t p e", p=P)

    consts = ctx.enter_context(tc.tile_pool(name="consts", bufs=1))
    dramp = ctx.enter_context(tc.tile_pool(name="scratch", bufs=1, space="DRAM"))

    ones_row = consts.tile([1, L], BF16)
    nc.vector.memset(ones_row[:], 1.0)
    epsv = consts.tile([1, 65], BF16)
    nc.vector.memset(epsv[:], 0.0)
    nc.vector.memset(epsv[:, 64:65], 1e-20)
    eps_ln = consts.tile([P, 1], F32)
    nc.vector.memset(eps_ln[:], LN_EPS)
    onescol = consts.tile([1, P], BF16)
    nc.vector.memset(onescol[:], 1.0)
    bvr = consts.tile([1, FL], BF16)
    sync.dma_start(bvr[:], bv_d.ap())
    bqc = consts.tile([P, FC], F32)
    sync.dma_start(bqc[:], bq_d.ap())
    bkc = consts.tile([P, FC], F32)
    sync.dma_start(bkc[:], bk_d.ap())
    wo_sb = consts.tile([P, FC, E], BF16)
    sync.dma_start(wo_sb[:], wo_d.ap())

    xhatT = consts.tile([P, EC, L], BF16)   # xhat transposed: [e, l]
    qT = consts.tile([P, FC, L], BF16)      # Q^T: [f, l] (scale folded in)
    kT = consts.tile([P, FC, L], BF16)      # K^T: [f, l]
    vaug = consts.tile([P, LC, HPC, 65], BF16)  # V | ones column, per l-chunk/head
    otall = consts.tile([P, FC, L], BF16)   # normalized attention output^T
    nc.vector.memset(vaug[:, :, :, 64:65], 1.0)

    xhat_dram = dramp.tile([L, E], BF16)
    qs_dram = dramp.tile([HPC, L], F32)

    # ---- Phase A: LayerNorm (natural layout), then DMA round-trip transpose ----
    with tc.tile_pool(name="xin", bufs=3) as xpool, \
         tc.tile_pool(name="stats", bufs=6) as statp, \
         tc.tile_pool(name="xh", bufs=3) as xhp:
        for t in range(LC):
            xt = xpool.tile([P, E], F32)
            sync.dma_start(xt[:], x_t[t])
            st = statp.tile([P, 2, 6], F32)
            nc.vector.bn_stats(st[:, 0, :], xt[:, 0:512])
            nc.vector.bn_stats(st[:, 1, :], xt[:, 512:1024])
            mv = statp.tile([P, 2], F32)
            nc.vector.bn_aggr(mv[:], st[:])
            srt = statp.tile([P, 1], F32)
            nc.scalar.activation(srt[:], mv[:, 1:2], AF.Sqrt, bias=eps_ln[:], scale=1.0)
            rstd = statp.tile([P, 1], F32)
            nc.vector.reciprocal(rstd[:], srt[:])
            xh = xhp.tile([P, E], BF16)
            nc.vector.tensor_scalar(xh[:], xt[:], mv[:, 0:1], rstd[:],
                                    op0=ALU.subtract, op1=ALU.mult)
            sync.dma_start(xhat_dram[bass.ts(t, P), :], xh[:])
    for et in range(EC):
        sync.dma_start(xhatT[:, et, :], xhat_dram[:, bass.ts(et, P)], transpose=True)

    # ---- Phase B: Q/K/V projections ----
    with tc.tile_pool(name="w", bufs=1) as wpool, \
         tc.tile_pool(name="pjqk", bufs=3, space="PSUM") as pjqk, \
         tc.tile_pool(name="pjv", bufs=2, space="PSUM") as pjv:
        wq_sb = wpool.tile([P, EC, FL], BF16)
        sync.dma_start(wq_sb[:], wq_d.ap())
        wk_sb = wpool.tile([P, EC, FL], BF16)
        sync.dma_start(wk_sb[:], wk_d.ap())
        wv_sb = wpool.tile([P, EC, FL], BF16)
        sync.dma_start(wv_sb[:], wv_d.ap())

        for fc in range(FC):
            for w_sb, dest, bcol in ((wq_sb, qT, bqc), (wk_sb, kT, bkc)):
                ps = pjqk.tile([P, L], F32)
                for half in range(2):
                    for ec in range(EC):
                        nc.tensor.matmul(
                            ps[:, half * 512:(half + 1) * 512],
                            lhsT=w_sb[:, ec, fc * P:(fc + 1) * P],
                            rhs=xhatT[:, ec, half * 512:(half + 1) * 512],
                            start=(ec == 0), stop=(ec == EC - 1))
                nc.scalar.activation(dest[:, fc, :], ps[:], AF.Identity,
                                     bias=bcol[:, fc:fc + 1], scale=1.0)
        for lc in range(LC):
            ps = pjv.tile([P, FL], F32)
            nc.tensor.matmul(ps[:], lhsT=onescol[:], rhs=bvr[:], start=True, stop=False)
            for ec in range(EC):
                nc.tensor.matmul(ps[:], lhsT=xhatT[:, ec, bass.ts(lc, P)],
                                 rhs=wv_sb[:, ec, :],
                                 start=False, stop=(ec == EC - 1))
            nc.vector.tensor_copy(vaug[:, lc, :, 0:64],
                                  ps[:].rearrange("p (h d) -> p h d", h=HPC))

    # ---- Phase C: attention, one head at a time, transposed layout ----
    with tc.tile_pool(name="egb", bufs=10) as egbp, \
         tc.tile_pool(name="attn", bufs=4) as atp, \
         tc.tile_pool(name="rows", bufs=4) as rowp, \
         tc.tile_pool(name="qsb", bufs=3) as qsbp, \
         tc.tile_pool(name="lg", bufs=2, space="PSUM") as lg, \
         tc.tile_pool(name="otp", bufs=2, space="PSUM") as otp:
        for h in range(HPC):
            fc, po = h // 2, (h % 2) * 64
            ot_ps = otp.tile([65, L], F32)
            for half in range(2):
                nc.tensor.matmul(ot_ps[:, half * 512:(half + 1) * 512],
                                 lhsT=epsv[:],
                                 rhs=ones_row[:, half * 512:(half + 1) * 512],
                                 start=True, stop=False)
            for kc in range(LC):
                egbt = egbp.tile([P, L], BF16)
                sync.dma_start(egbt[:], eg_d.ap()[h, kc])
                lgt = lg.tile([P, L], F32)
                for half in range(2):
                    nc.tensor.matmul(
                        lgt[:, half * 512:(half + 1) * 512],
                        lhsT=kT[po:po + 64, fc, bass.ts(kc, P)],
                        rhs=qT[po:po + 64, fc, half * 512:(half + 1) * 512],
                        start=True, stop=True)
                el = atp.tile([P, L], BF16, tag="el")
                nc.scalar.activation(el[:], lgt[:], AF.Exp)
                at = atp.tile([P, L], BF16, tag="at")
                nc.vector.tensor_mul(at[:], el[:], egbt[:])
                for half in range(2):
                    nc.tensor.matmul(
                        ot_ps[:, half * 512:(half + 1) * 512],
                        lhsT=vaug[:, kc, h, :],
                        rhs=at[:, half * 512:(half + 1) * 512],
                        start=False, stop=(kc == LC - 1))
            # normalize columns by the ones-row sum (and broadcast via DRAM round-trip)
            qs = rowp.tile([1, L], F32)
            nc.vector.reciprocal(qs[:], ot_ps[64:65, :])
            sync.dma_start(qs_dram[h:h + 1, :], qs[:])
            qsb = qsbp.tile([64, L], F32)
            qap = qs_dram[h:h + 1, :]
            bc = bass.AP(tensor=qap.tensor, offset=qap.offset,
                         ap=[[0, 64], qap.ap[-1]])
            nc.gpsimd.dma_start(qsb[:], bc)
            nc.vector.tensor_tensor(otall[po:po + 64, fc, :], ot_ps[0:64, :],
                                    qsb[:], op=ALU.mult)

    # ---- Phase D: output projection (partial, host adds residual+bo and pairs) ----
    with tc.tile_pool(name="op", bufs=2, space="PSUM") as op, \
         tc.tile_pool(name="outs", bufs=3) as outp:
        for lc in range(LC):
            ps = op.tile([P, E], F32)
            for half in range(2):
                for fc in range(FC):
                    nc.tensor.matmul(
                        ps[:, half * 512:(half + 1) * 512],
                        lhsT=otall[:, fc, bass.ts(lc, P)],
                        rhs=wo_sb[:, fc, half * 512:(half + 1) * 512],
                        start=(fc == 0), stop=(fc == FC - 1))
            ot = outp.tile([P, E], F32)
            nc.scalar.copy(ot[:, 0:512], ps[:, 0:512])
            nc.vector.tensor_copy(ot[:, 512:1024], ps[:, 512:1024])
            sync.dma_start(out_t[lc], ot[:])


def build_nc():
    nc = bacc.Bacc("TRN2", target_bir_lowering=False, debug=False)
    xd = nc.dram_tensor("x", [L, E], F32, kind="ExternalInput")
    wq_d = nc.dram_tensor("wqT", [P, EC, FL], BF16, kind="ExternalInput")
    wk_d = nc.dram_tensor("wkT", [P, EC, FL], BF16, kind="ExternalInput")
    wv_d = nc.dram_tensor("wvT", [P, EC, FL], BF16, kind="ExternalInput")
    wo_d = nc.dram_tensor("woT", [P, FC, E], BF16, kind="ExternalInput")
    bq_d = nc.dram_tensor("bqc", [P, FC], F32, kind="ExternalInput")
    bk_d = nc.dram_tensor("bkc", [P, FC], F32, kind="ExternalInput")
    bv_d = nc.dram_tensor("bvr", [1, FL], BF16, kind="ExternalInput")
    eg_d = nc.dram_tensor("egb", [HPC, LC, P, L], BF16, kind="ExternalInput")
    out_d = nc.dram_tensor("partial", [L, E], F32, kind="ExternalOutput")
    with tile.TileContext(nc) as tc, ExitStack() as ctx:
        _emit(nc, tc, ctx, xd, wq_d, wk_d, wv_d, wo_d, bq_d, bk_d, bv_d, eg_d, out_d)
    nc.compile()
    return nc


def _wdev(w):
    # [FL, E] slice of an LN-folded weight -> lhsT layout [P, EC, FL]
    return np.ascontiguousarray(
        w.T.reshape(EC, P, FL).transpose(1, 0, 2)).astype(NBF16)


def prepare_in_maps(x, bias, mask, Wq, bq, Wk, bk, Wv, bv, Wo, bo, gamma, beta, gate):
    x = np.asarray(x, np.float32)
    gamma = np.asarray(gamma, np.float32)
    beta = np.asarray(beta, np.float32)
    gate = np.asarray(gate, np.float32)
    Wq = np.asarray(Wq, np.float32)
    Wk = np.asarray(Wk, np.float32)
    Wv = np.asarray(Wv, np.float32)
    Wo = np.asarray(Wo, np.float32)
    bq = np.asarray(bq, np.float32)
    bk = np.asarray(bk, np.float32)
    bv = np.asarray(bv, np.float32)
    scale = 1.0 / np.sqrt(np.float32(D))

    Wqe = (Wq * gamma[None, :]) * scale
    Wke = Wk * gamma[None, :]
    Wve = Wv * gamma[None, :]
    bqe = (bq + Wq @ beta) * scale
    bke = bk + Wk @ beta
    bve = bv + Wv @ beta
    mf = np.asarray(mask, np.float32)

    in_maps = []
    for c in range(NCORES):
        b, h0 = c // 2, (c % 2) * HPC
        sl = slice(h0 * D, h0 * D + FL)
        g = gate[h0:h0 + HPC]
        bb = np.asarray(bias[b, h0:h0 + HPC], np.float32)      # [HPC, q, k]
        egb = np.exp(g[:, None, None] * bb)
        egb *= mf[b][None, None, :]                            # key mask
        egb *= mf[b][None, :, None]                            # query mask
        egbT = np.ascontiguousarray(egb.transpose(0, 2, 1)).reshape(HPC, LC, P, L)
        in_maps.append({
            "x": np.ascontiguousarray(x[b]),
            "wqT": _wdev(Wqe[sl]),
            "wkT": _wdev(Wke[sl]),
            "wvT": _wdev(Wve[sl]),
            "woT": np.ascontiguousarray(
                Wo[:, sl].T.reshape(FC, P, E).transpose(1, 0, 2)).astype(NBF16),
            "bqc": np.ascontiguousarray(bqe[sl].reshape(FC, P).T),
            "bkc": np.ascontiguousarray(bke[sl].reshape(FC, P).T),
            "bvr": bve[sl].reshape(1, FL).astype(NBF16),
            "egb": egbT.astype(NBF16),
        })
    return in_maps


def finish(x, bo, partials):
    x = np.asarray(x, np.float32)
    bo = np.asarray(bo, np.float32)
    out = np.empty((B, L, E), np.float32)
    for b in range(B):
        out[b] = x[b] + partials[2 * b] + partials[2 * b + 1] + bo[None, :]
    return out


def run_spmd(in_maps, trace=False, trace_cores=None):
    global _NC
    if _NC is None:
        _NC = build_nc()
    return run_bass_kernel_spmd(_NC, in_maps, core_ids=list(range(NCORES)),
                                trace=trace, trace_cores=trace_cores)


def kernel(**inputs):
    in_maps = prepare_in_maps(**inputs)
    res = run_spmd(in_maps)
    partials = [r["partial"] for r in res.results]
    return finish(inputs["x"], inputs["bo"], partials)
